# revision 73
# baseline (speedup 1.0000x reference)
"""Causal multi-head attention (B=4, S=2048, D=512, H=8) on 8 trn2 cores.

Sharding: core c handles batch b = c//2 and head-group g = c%2 (4 heads).
Host pre-transposes activations into chunk-major contiguous blocks, casts
weights to bf16, and sums the two head-group partial outputs per batch
(the W_O row-parallel reduce).

Device kernel (per core), matmuls bf16 with f32 PSUM accumulation:
  QT/KT = W.T-slices @ x.T          [d=256, S]   (d on partitions)
  V     = x @ Wv.T-slice            packed as [V_h(64) | ones(64)] blocks
  ST    = K_h^T.T @ Q_h^T           [k, q] per 128-k-tile, block-causal,
                                    head pair on PE row groups 0:64/64:128
  P     = exp(ST/8)   split between ACT (spline exp) and DVE (Schraudolph
                      bitcast exp) in a DDAA period-4 pattern so the
                      st-psum rotation chain alternates engines
  pv    = [V_h|1].T @ P             M=128 -> [ctx_h(64); den_h(64)] per head
  rec   = exp(-ln(den)) on ACT (ln/exp share one table set)
  ctx   = pv_ctx * rec              DVE
  out.T = Wo-slice.T @ ctx          [512, S] bf16 partial (host adds pairs)

Emission is software-pipelined: PV for tile i-CARRY_LAG issues after the
ST/exp of tile i so the PE never stalls on one tile's exp; projections
and out-projections are interleaved between attention blocks as PE
filler; dummy matmuls keep the PE HAM clock-gate warm during the
DMA-bound head of the kernel.
"""
import sys

sys.path.insert(0, "/opt/trn_rl_repo")
from contextlib import ExitStack

import numpy as np
import ml_dtypes

import bass_rust
import concourse.bass as bass
import concourse.tile as tile
from concourse import mybir
from concourse.bass_utils import run_bass_kernel_spmd
from concourse.vector_clock import ScopedClock

BF16 = mybir.dt.bfloat16
F32 = mybir.dt.float32
I32 = mybir.dt.int32
I16 = mybir.dt.int16
F8 = mybir.dt.float8e4
DR = mybir.MatmulPerfMode.DoubleRow
EXP = mybir.ActivationFunctionType.Exp
LOG = mybir.ActivationFunctionType.Ln
MUL = mybir.AluOpType.mult
ADD = mybir.AluOpType.add

B, S, D, H = 4, 2048, 512, 8
# Schraudolph fast-exp on DVE: bf16bits(exp(x/8)) ~ i16((x*SCH_A + SCH_B)/65536)
SCH_A = 0.125 * (2**23) / float(np.log(2.0))
SCH_B = 1064987000.0
DK = 64          # head dim
HG = 2           # head groups (cores per batch)
NF = 4           # 128-rows tiles of the contraction dim D
NK = 16          # 128-wide k tiles
NJ = 4           # 512-wide q blocks
N_CORES = 8
WAITS_WIDE = 1

# knobs
EXP_DVE_OF_8 = 5   # of every 8 score tiles, this many exp on DVE
PG_BUFS = 7
CARRY_LAG = 5
ST_BUFS = 2
PV_BUFS = 4
# fp8e4m3 Q/K + DoubleRow perf mode for the score matmuls: measured on HW
# this gives NO matmul speedup (683ns vs 386ns bf16 for the same tile) and
# rel err 2.4e-2 > 2e-2 gate — keep disabled
ST_FP8 = False

# ---------------------------------------------------------------------------
# Workarounds for this walrus build: at most ONE sync wait per instruction.
_ctr = [0]


class _TC(tile.TileContext):
    def _drain_and_barrier(self, tick_clock, wait_clock):
        nc = self.nc
        drain_inst = nc.sync.drain()
        wait_clock.add_sem_waits(
            drain_inst.ins, ScopedClock({None: tick_clock.global_clock})
        )
        si = drain_inst.ins.sync_info
        waits = list(si.on_wait) if si is not None else []
        if waits:
            drain_inst.ins.sync_info = bass_rust.SyncInfo(
                on_wait=[], on_update=list(si.on_update)
            )
            for w in waits:
                nop = nc.sync.nop(nofuse=True)
                nop.ins.sync_info = bass_rust.SyncInfo(on_wait=[w], on_update=[])
        nc.all_engine_barrier()
        assert self.sems is not None
        popped = nc._tile_sem_poison_stack.pop()
        assert popped is self._sem_poison
        nc.clear_and_free_semaphores(list(self.sems.allocated().values()))
        # no trailing all_engine_barrier: the gpsimd sem clears are the last
        # instructions and the runtime waits for every engine queue to drain
        # before reporting completion, so the barrier only adds tail latency


def _fix_sync_waits(nc, maxw=1):
    for f in nc.m.functions:
        for bb in f.blocks:
            insts = list(bb.instructions)
            out = []
            dirty = False
            for inst in insts:
                si = inst.sync_info
                if si is not None:
                    waits = list(si.on_wait)
                    if isinstance(inst, mybir.InstDrain):
                        limit = 0
                    elif isinstance(inst, (mybir.InstMatmult, mybir.InstLdweights, mybir.InstActivation, mybir.InstTensorScalarPtr, mybir.InstTensorTensor)):
                        limit = WAITS_WIDE
                    else:
                        limit = maxw
                    if len(waits) > limit:
                        keep, extra = waits[:limit], waits[limit:]
                        for i in range(0, len(extra), maxw):
                            _ctr[0] += 1
                            nop = mybir.InstNoOp(name=f"ws-{_ctr[0]}")
                            nop.engine = inst.engine
                            nop.sync_info = bass_rust.SyncInfo(
                                on_wait=extra[i : i + maxw], on_update=[]
                            )
                            out.append(nop)
                        inst.sync_info = bass_rust.SyncInfo(
                            on_wait=keep, on_update=list(si.on_update)
                        )
                        dirty = True
                out.append(inst)
            if dirty:
                bb.instructions = out


# ---------------------------------------------------------------------------
def _make_pools(ctx, tc):
    return dict(
        cpool=ctx.enter_context(tc.tile_pool(name="const", bufs=1)),
        stp=ctx.enter_context(tc.tile_pool(name="stp", bufs=2 * ST_BUFS, space="PSUM")),
        # pv2 accumulators and the proj/out psums live in separate 1-slot
        # pools (2 banks each): the proj stream is woven into the attention
        # blocks as fillers, so its serialized evacuations hide behind
        # attention tiles instead of head-blocking the next block's exps
        pvp=ctx.enter_context(tc.tile_pool(name="pvp", bufs=1, space="PSUM")),
        projp=ctx.enter_context(tc.tile_pool(name="projp", bufs=1, space="PSUM")),
        ppool=ctx.enter_context(tc.tile_pool(name="ppool", bufs=PG_BUFS)),
        rpool=ctx.enter_context(tc.tile_pool(name="rpool", bufs=2)),
        opool=ctx.enter_context(tc.tile_pool(name="opool", bufs=2)),
        qpool=ctx.enter_context(tc.tile_pool(name="qpool", bufs=2)),
    )


def _emit_body(nc, tc, aps, pools, tag=""):
    xq, xk, xv, wq, wk, wv, wo, tri, outT = aps
    cpool = pools["cpool"]
    stp = pools["stp"]
    pvp = pools["pvp"]
    projp = pools["projp"]
    ppool = pools["ppool"]
    rpool = pools["rpool"]
    opool = pools["opool"]

    def ctile(shape, dtype, t):
        return cpool.tile(shape, dtype, tag=t + tag, name=t + tag)

    # ---- const tiles (f-merged so each input needs one DMA per chunk:
    # the sync engine pays ~600ns of descriptor issue per dma_start)
    wq_a = ctile([128, 1024], BF16, "wqa")   # f block at cols 256f
    wk_a = ctile([128, 1024], BF16, "wka")
    wv_a = ctile([128, 1024], BF16, "wva")
    xq_a = ctile([128, 4 * S], BF16, "xqa")  # f block at cols 2048f
    xk_a = ctile([128, 4 * S], BF16, "xka")
    xv_a = ctile([128, 4 * S], BF16, "xva")
    wo_a = ctile([128, 1024], BF16, "woa")  # t2 block at cols 512t2
    tri_t = ctile([128, 128], BF16, "tri")
    ones_t = ctile([128, 64], BF16, "ones")
    if ST_FP8:
        # per head h: a [32, (qk:2)x(i:2)x(s:S)] fp8 block, i = high/low
        # half of the 64-dim head (DoubleRow fold).  Matmul operand APs
        # only allow partition bases {0,32,64}, so head 3 shares base 64
        # with head 2 in a disjoint column range.
        QK8_PB = (0, 32, 64, 64)
        QK8_CB = (0, 0, 0, 4 * S)
        qk8 = ctile([128, 8 * S], F8, "qk8")
        qk_t = None
    else:
        # merged Q^T/K^T per head pair: qt at cols 0:S, kt at S:2S
        qk_t = [ctile([128, 2 * S], BF16, f"QK{t2}") for t2 in range(2)]
    # vhat: per k-tile i a [128, 512] block of 4 heads x [V_h(64) | ones(64)]
    vhat = ctile([128, NK * 512], BF16, "vhat")
    # merged ctx per J: head-pair t2 block at cols 512*t2
    ctx_j = [ctile([128, 1024], BF16, f"ctx{J}") for J in range(NJ)]

    def wslice(t, f, lo, hi):
        return t[:, 256 * f + lo : 256 * f + hi]

    def xslice(t, c, f, lo, hi):
        # x tiles are chunk-major: chunk c at cols 2048c, f-block at 512f
        return t[:, 2048 * c + 512 * f + lo : 2048 * c + 512 * f + hi]

    # ---- PE warmup: dummy matmuls on a memset tile spin the HAM clock gate
    # to K=8/8 while the input DMAs stream in.
    wup = ctile([128, 512], BF16, "wup")
    nc.gpsimd.memset(ones_t[:], 1.0)
    nc.gpsimd.memset(wup[:], 0.0)
    # ones blocks of vhat (cols 64:128 of each 128-block), set once
    nc.gpsimd.memset(
        vhat[:].rearrange("p (b w) -> p b w", w=128)[:, :, 64:128], 1.0
    )
    wps = stp.tile([128, 512], F32, tag="st", name=f"wps{tag}")
    for _ in range(12):
        nc.tensor.matmul(wps[:], wup[:, 0:128], wup[:], start=True, stop=True)

    # ---- DMA emission in consumption order.
    # Host arrays are pre-laid per-partition-contiguous (rows = partitions,
    # cols = the full sbuf free range), so every DMA is a plain 2D copy
    # with 2-4KB descriptors instead of 512B-1KB ones.
    def dma_w(t, ap, f, eng=None):
        (eng or nc.sync).dma_start(t[:], ap)

    def dma_x_chunk(t, ap, c, eng, split=1):
        for s in range(split):
            w = 2048 // split
            eng.dma_start(
                t[:, 2048 * c + w * s : 2048 * c + w * (s + 1)],
                ap[128 * c : 128 * (c + 1), w * s : w * (s + 1)],
            )

    # first-chunk DMAs issued from two engines in parallel so the critical
    # wq+xq0+wk+xk0 set is all in flight sooner
    dma_w(wq_a, wq, 4)
    dma_x_chunk(xq_a, xq, 0, nc.sync)
    dma_w(wk_a, wk, 4, nc.scalar)
    dma_x_chunk(xk_a, xk, 0, nc.scalar)
    nc.scalar.dma_start(tri_t[:], tri[:])
    dma_w(wv_a, wv, 4)
    dma_x_chunk(xv_a, xv, 0, nc.sync)
    for c in range(1, 4):
        dma_x_chunk(xq_a, xq, c, nc.sync)
        dma_x_chunk(xk_a, xk, c, nc.sync)
        if c == 1:
            dma_w(wo_a, wo, 2)
        dma_x_chunk(xv_a, xv, c, nc.sync)

    # ---- emit helpers -----------------------------------------------------
    # proj / V / out-proj emissions are chunked into closures ("fillers")
    # that the schedule weaves between attention tiles, so their psum
    # evacuations interleave with exps in the ACT/DVE queues instead of
    # head-blocking a block's first exps.
    def qk_fillers(t2, J):
        box = {}

        def mm_half(half, w_a, x_a):
            def f():
                if "ps" not in box:
                    box["ps"] = projp.tile(
                        [128, 1024], F32, tag="pj", name=f"psqk{t2}_{J}{tag}"
                    )
                    if J == 0:
                        # PE/HAM warmup while the input DMAs stream
                        for _ in range(3):
                            nc.tensor.matmul(
                                box["ps"][:, 0:512], wup[:, 0:128], wup[:],
                                start=True, stop=True,
                            )
                ps = box["ps"]
                for ff in range(NF):
                    nc.tensor.matmul(
                        ps[:, 512 * half : 512 * (half + 1)],
                        wslice(w_a, ff, 128 * t2, 128 * (t2 + 1)),
                        xslice(x_a, J, ff, 0, 512),
                        start=(ff == 0),
                        stop=(ff == NF - 1),
                    )
            return f

        def copy_q():
            ps = box["ps"]
            dst = qk_t[t2][:].rearrange("p (h s) -> p h s", h=2)[:, :, 512 * J : 512 * (J + 1)]
            nc.scalar.copy(dst[:, 0:1, :], ps[:, 0:512].rearrange("p (h w) -> p h w", h=1))

        def copy_k():
            ps = box["ps"]
            dst = qk_t[t2][:].rearrange("p (h s) -> p h s", h=2)[:, :, 512 * J : 512 * (J + 1)]
            nc.vector.tensor_copy(dst[:, 1:2, :], ps[:, 512:1024].rearrange("p (h w) -> p h w", h=1))

        return [mm_half(0, wq_a, xq_a), mm_half(1, wk_a, xk_a), copy_q, copy_k]

    def v_fillers(c):
        # V for k-tiles i in [4c, 4c+4): one [128,1024] psum (4 x 256),
        # strided casts into the V-halves of vhat's [V|1] blocks
        box = {}

        def mm_pair(lo):
            def f():
                if "ps" not in box:
                    box["ps"] = projp.tile([128, 1024], F32, tag="pj", name=f"psv{c}{tag}")
                    if c == 0:
                        for _ in range(3):
                            nc.tensor.matmul(
                                box["ps"][:, 0:512], wup[:, 0:128], wup[:],
                                start=True, stop=True,
                            )
                ps = box["ps"]
                for ii in range(lo, lo + 2):
                    for ff in range(NF):
                        nc.tensor.matmul(
                            ps[:, 256 * ii : 256 * (ii + 1)],
                            xslice(xv_a, c, ff, 128 * ii, 128 * (ii + 1)),
                            wslice(wv_a, ff, 0, 256),
                            start=(ff == 0),
                            stop=(ff == NF - 1),
                        )
            return f

        def cast_lo():
            dst = vhat[:, 2048 * c : 2048 * (c + 1)].rearrange("p (b w) -> p b w", w=128)
            nc.vector.tensor_copy(
                dst[:, 0:8, 0:64], box["ps"][:, 0:512].rearrange("p (b w) -> p b w", w=64)
            )

        def cast_hi():
            dst = vhat[:, 2048 * c : 2048 * (c + 1)].rearrange("p (b w) -> p b w", w=128)
            nc.scalar.copy(
                dst[:, 8:16, 0:64], box["ps"][:, 512:1024].rearrange("p (b w) -> p b w", w=64)
            )

        return [mm_pair(0), mm_pair(2), cast_lo, cast_hi]

    ec = [0]
    carry = []  # global pending PV-flush closures (pipeline depth 2)

    def step_carry():
        carry.pop(0)()

    def drain_carry():
        while carry:
            step_carry()

    def emit_att(J, hp, fill=None, lag=CARRY_LAG):
        fill = fill if fill is not None else []
        nk = 4 * (J + 1)
        ha, hb = 2 * hp, 2 * hp + 1
        qk = None if ST_FP8 else qk_t[hp]

        def st_mm(h, out_ap, i, qlo):
            pb, cb = QK8_PB[h], QK8_CB[h]
            t = qk8[pb : pb + 32, cb : cb + 4 * S].rearrange("p (qk i s) -> p qk i s", qk=2, i=2)
            nc.tensor.matmul(
                out_ap,
                t[:, 1, :, 128 * i : 128 * (i + 1)],
                t[:, 0, :, 512 * J + qlo : 512 * (J + 1)],
                start=True, stop=True, perf_mode=DR,
            )
        # pv2: head a at cols 0:512, head b at 512:1024; each [ctx(64); den(64)]
        pv2 = pvp.tile([128, 1024], F32, tag="pv", name=f"pv{J}_{hp}{tag}")

        def emit_recip():
            # Copy pv2 out to SBUF immediately (releases the PSUM bank pair
            # for the next block), then 1/den = exp(-log(den)) on ACT and
            # per head ctx = pv_ctx * rec on DVE from the copy.  The final
            # block is latency-critical (nothing left to overlap), so its
            # two heads run on ACT and DVE concurrently.
            lg = rpool.tile([64, 1024], F32, tag="rect", name=f"rect{J}_{hp}{tag}")
            rec = rpool.tile([64, 1024], F32, tag="rec", name=f"rec{J}_{hp}{tag}")
            cj = ctx_j[J][:, 512 * hp : 512 * (hp + 1)]
            if (J == NJ - 1) and (hp == 1):
                # tail: read psum directly, heads split ACT / DVE-Newton
                den = pv2[64:128, :]
                nc.scalar.activation(lg[:, 0:512], den[:, 0:512], LOG)
                nc.scalar.activation(rec[:, 0:512], lg[:, 0:512], EXP, scale=-1.0)
                nc.vector.tensor_scalar(
                    rec[:, 512:1024].bitcast(I32), den[:, 512:1024].bitcast(I32),
                    -1, 0x7EF311C3, MUL, ADD,
                )
                nc.vector.tensor_mul(lg[:, 512:1024], den[:, 512:1024], rec[:, 512:1024])
                nc.vector.scalar_tensor_tensor(
                    lg[:, 512:1024], lg[:, 512:1024], 2.0, rec[:, 512:1024],
                    mybir.AluOpType.subtract, MUL,
                )
                nc.vector.tensor_mul(cj[0:64, :], pv2[0:64, 0:512], rec[:, 0:512])
                nc.vector.scalar_tensor_tensor(
                    cj[64:128, :], pv2[0:64, 512:1024], -1.0, lg[:, 512:1024],
                    MUL, MUL,
                )
            else:
                nc.scalar.activation(lg[:], pv2[64:128, :], LOG)
                nc.scalar.activation(rec[:], lg[:], EXP, scale=-1.0)
                nc.vector.tensor_mul(cj[0:64, :], pv2[0:64, 0:512], rec[:, 0:512])
                nc.vector.tensor_mul(cj[64:128, :], pv2[0:64, 512:1024], rec[:, 512:1024])

        def make_flush(i, pg, qlo, st_, sp_):
            def fl():
                va = vhat[:, 512 * i + 128 * ha : 512 * i + 128 * (ha + 1)]
                vb = vhat[:, 512 * i + 128 * hb : 512 * i + 128 * (hb + 1)]
                nc.tensor.matmul(
                    pv2[:, qlo:512], va, pg[:, qlo:512], start=st_, stop=sp_,
                )
                nc.tensor.matmul(
                    pv2[:, 512 + qlo : 1024], vb, pg[:, 512 + qlo : 1024],
                    start=st_, stop=sp_,
                )
                if sp_:
                    emit_recip()
            return fl

        # tile 0 first (full-width, unmasked: the PSUM-start flush never
        # waits on the gpsimd mask chain), then diagonal (masked) tiles so
        # their exp->mask chain gets CARRY_LAG tiles of slack, then the
        # remaining off-diagonal tiles so the block-end carry drain flushes
        # tiles whose pg needs no gpsimd hop.
        if J == 0:
            order = [0, 1, 2, 3]
        else:
            order = [0] + list(range(4 * J, nk)) + list(range(1, 4 * J))
        for idx, i in enumerate(order):
            dq = i - 4 * J
            qlo = 128 * dq if dq > 0 else 0
            # per-head score tiles: halves the exp latency in the stp ring
            # (DVE exps head a, ACT exps head b, concurrently on every tile)
            sta = stp.tile([128, 512], F32, tag="st", name=f"sta{J}_{hp}_{i}{tag}")
            stb = stp.tile([128, 512], F32, tag="st", name=f"stb{J}_{hp}_{i}{tag}")
            pg = ppool.tile([128, 1024], BF16, tag="pg", name=f"pg{J}_{hp}_{i}{tag}")
            if ST_FP8:
                st_mm(ha, sta[:, qlo:512], i, qlo)
                st_mm(hb, stb[:, qlo:512], i, qlo)
            else:
                nc.tensor.matmul(
                    sta[:, qlo:512],
                    qk[0:64, S + 128 * i : S + 128 * (i + 1)],
                    qk[0:64, 512 * J + qlo : 512 * (J + 1)],
                    start=True, stop=True,
                )
                nc.tensor.matmul(
                    stb[:, qlo:512],
                    qk[64:128, S + 128 * i : S + 128 * (i + 1)],
                    qk[64:128, 512 * J + qlo : 512 * (J + 1)],
                    start=True, stop=True,
                )
            nc.vector.tensor_scalar(
                pg[:, qlo:512].bitcast(I16), sta[:, qlo:512],
                SCH_A / 65536.0, SCH_B / 65536.0, MUL, ADD,
            )
            if idx <= 1:
                # the previous block's reciprocal (~1.8us) heads the ACT
                # queue at block start; the first two tiles' head-b exps go
                # to DVE so the block's first PV flushes never wait on that
                # backlog
                nc.vector.tensor_scalar(
                    pg[:, 512 + qlo : 1024].bitcast(I16), stb[:, qlo:512],
                    SCH_A / 65536.0, SCH_B / 65536.0, MUL, ADD,
                )
            else:
                nc.scalar.activation(
                    pg[:, 512 + qlo : 1024], stb[:, qlo:512], EXP, scale=0.125
                )
            if dq >= 0:
                nc.gpsimd.tensor_mul(pg[:, qlo : qlo + 128], pg[:, qlo : qlo + 128], tri_t[:])
                nc.gpsimd.tensor_mul(
                    pg[:, 512 + qlo : 512 + qlo + 128],
                    pg[:, 512 + qlo : 512 + qlo + 128], tri_t[:],
                )
            if len(carry) >= lag:
                step_carry()
            carry.append(make_flush(i, pg, qlo, idx == 0, idx == nk - 1))
            # weave in pending proj/out work between attention tiles; short
            # blocks pop faster so fewer fillers drain in a burst at block
            # end (right where the next block's exps queue)
            if idx >= 1:
                for _ in range(2 if nk > 8 else 3):
                    if fill:
                        fill.pop(0)()
        drain_carry()
        # anything not absorbed must still be emitted before the next block
        # (the dependency tracker keys on emission order)
        while fill:
            fill.pop(0)()

    def po_mm(po, J, mp):
        for mh in range(2):
            m = 2 * mp + mh
            for t2 in range(2):
                nc.tensor.matmul(
                    po[:, 512 * mh : 512 * (mh + 1)],
                    wo_a[:, 512 * t2 + 128 * m : 512 * t2 + 128 * (m + 1)],
                    ctx_j[J][:, 512 * t2 : 512 * (t2 + 1)],
                    start=(t2 == 0), stop=(t2 == 1),
                )

    def out_fillers(J):
        box = {}
        out3 = outT[512 * J : 512 * (J + 1), :].rearrange("(m p) w -> p m w", m=4)

        def mm(mp):
            def f():
                if "ob" not in box:
                    box["ob"] = opool.tile([128, 2048], BF16, tag="ob", name=f"ob{J}{tag}")
                box[mp] = projp.tile([128, 1024], F32, tag="pj", name=f"po{J}_{mp}{tag}")
                po_mm(box[mp], J, mp)
            return f

        def cp(mp):
            def f():
                ob = box["ob"]
                if mp == 0:
                    nc.scalar.copy(ob[:, 0:1024], box[0][:])
                else:
                    nc.vector.tensor_copy(ob[:, 1024:2048], box[1][:])
            return f

        def dma(mp):
            def f():
                (nc.scalar if mp == 0 else nc.sync).dma_start(
                    out3[:, 2 * mp : 2 * mp + 2, :],
                    box["ob"][:, 1024 * mp : 1024 * (mp + 1)].rearrange("p (m w) -> p m w", m=2),
                )
            return f

        return [mm(0), cp(0), dma(0), mm(1), cp(1), dma(1)]

    def emit_out_last(J):
        # tail is latency-critical: copy halves on both engines in
        # parallel and fire each half's DMA as soon as it lands
        ob = opool.tile([128, 2048], BF16, tag="ob", name=f"ob{J}{tag}")
        out3 = outT[512 * J : 512 * (J + 1), :].rearrange("(m p) w -> p m w", m=4)
        for mp in range(2):
            # mp=1 reuses pv2's pool: its slot frees exactly when the final
            # recip (the tail dependency anyway) finishes reading pv2
            pool = projp if mp == 0 else pvp
            po = pool.tile([128, 1024], F32, tag="pj" if mp == 0 else "pv", name=f"po{J}_{mp}{tag}")
            po_mm(po, J, mp)
            for mh in range(2):
                m = 2 * mp + mh
                cp_ = nc.scalar.copy if mh == 0 else nc.vector.tensor_copy
                cp_(
                    ob[:, 1024 * mp + 512 * mh : 1024 * mp + 512 * (mh + 1)],
                    po[:, 512 * mh : 512 * (mh + 1)],
                )
                (nc.scalar if mh else nc.sync).dma_start(
                    out3[:, m : m + 1, :],
                    ob[:, 1024 * mp + 512 * mh : 1024 * mp + 512 * (mh + 1)].rearrange(
                        "p (m w) -> p m w", m=1
                    ),
                )

    # ---- schedule: J=0 projections run up front (DMA-bound phase); all
    # later proj/V/out work is woven between attention tiles as fillers.
    for f in qk_fillers(0, 0) + qk_fillers(1, 0) + v_fillers(0):
        f()
    fill = []
    for J in range(NJ):
        if J > 0:
            fill += out_fillers(J - 1)
        emit_att(J, 0, fill)
        if J + 1 < NJ:
            fill += v_fillers(J + 1)
            fill += qk_fillers(0, J + 1)
            fill += qk_fillers(1, J + 1)
        # final block: shorter flush lag so the end-of-block drain is two
        # flush-pairs shorter and the last reciprocal starts ~0.8us earlier
        emit_att(J, 1, fill, lag=3 if J == NJ - 1 else CARRY_LAG)
    # dummy matmuls bridge the final-reciprocal PE gap so the HAM clock
    # gate stays at full rate for the last out-projection matmuls; 18 of
    # them because the trace still showed a 1.6us gap (and a clock-gate
    # drop) after 12
    wfin = stp.tile([128, 512], F32, tag="st", name=f"wfin{tag}")
    for _ in range(18):
        nc.tensor.matmul(wfin[:], wup[:, 0:128], wup[:], start=True, stop=True)
    emit_out_last(NJ - 1)


def build(repeat=1, dup=frozenset()):
    nc = bass.Bass("TRN2", target_bir_lowering=False, debug=False, num_devices=N_CORES)
    xq = nc.dram_tensor("xq", [512, 2048], BF16, kind="ExternalInput").ap()
    xk = nc.dram_tensor("xk", [512, 2048], BF16, kind="ExternalInput").ap()
    xv = nc.dram_tensor("xv", [512, 2048], BF16, kind="ExternalInput").ap()
    wq = nc.dram_tensor("wq", [128, 1024], BF16, kind="ExternalInput").ap()
    wk = nc.dram_tensor("wk", [128, 1024], BF16, kind="ExternalInput").ap()
    wv = nc.dram_tensor("wv", [128, 1024], BF16, kind="ExternalInput").ap()
    wo = nc.dram_tensor("wo", [128, 1024], BF16, kind="ExternalInput").ap()
    tri = nc.dram_tensor("tri", [128, 128], BF16, kind="ExternalInput").ap()
    outT = nc.dram_tensor("outT", [D * 4, 512], BF16, kind="ExternalOutput").ap()
    aps = (xq, xk, xv, wq, wk, wv, wo, tri, outT)
    with _TC(nc) as tc:
        with ExitStack() as ctx:
            pools = _make_pools(ctx, tc)
            for r in range(repeat):
                _emit_body(nc, tc, aps, pools, tag=(f"_r{r}" if r else ""))
    _fix_sync_waits(nc)
    return nc


def make_in_maps(input_Q, input_K, input_V, W_Q, W_K, W_V, W_O):
    bf = ml_dtypes.bfloat16

    def chunked(xTb):
        # [D, S] -> [4c*128p, (f w)]: chunk c at rows 128c, each partition
        # row holds its full 4KB of (f, w) data contiguously
        a = xTb.reshape(4, 128, 4, 512).transpose(2, 1, 0, 3)
        return np.ascontiguousarray(a.reshape(512, 2048))

    def wfold(wT, f):
        # [f*128, w] -> [128, f*w]: per-partition f-major contiguous
        a = wT.reshape(f, 128, -1).transpose(1, 0, 2)
        return np.ascontiguousarray(a.reshape(128, -1))

    xT = {}
    for b in range(B):
        xT[("q", b)] = chunked(input_Q[b].T.astype(bf))
        xT[("k", b)] = chunked(input_K[b].T.astype(bf))
        xT[("v", b)] = chunked(input_V[b].T.astype(bf))
    wslices = {}
    for g in range(HG):
        rows = slice(256 * g, 256 * (g + 1))
        wslices[("wq", g)] = wfold(np.ascontiguousarray(W_Q[rows, :].T).astype(bf), 4)
        wslices[("wk", g)] = wfold(np.ascontiguousarray(W_K[rows, :].T).astype(bf), 4)
        wslices[("wv", g)] = wfold(np.ascontiguousarray(W_V[rows, :].T).astype(bf), 4)
        wslices[("wo", g)] = wfold(np.ascontiguousarray(W_O[:, rows].T).astype(bf), 2)
    kk, qq = np.meshgrid(np.arange(128), np.arange(128), indexing="ij")
    tri_np = (qq >= kk).astype(bf)
    in_maps = []
    for c in range(N_CORES):
        b, g = c // HG, c % HG
        in_maps.append(
            {
                "xq": xT[("q", b)],
                "xk": xT[("k", b)],
                "xv": xT[("v", b)],
                "wq": wslices[("wq", g)],
                "wk": wslices[("wk", g)],
                "wv": wslices[("wv", g)],
                "wo": wslices[("wo", g)],
                "tri": tri_np,
            }
        )
    return in_maps


_cache = {}


def kernel(**inputs):
    input_Q = np.asarray(inputs["input_Q"], np.float32)
    input_K = np.asarray(inputs["input_K"], np.float32)
    input_V = np.asarray(inputs["input_V"], np.float32)
    W_Q = np.asarray(inputs["W_Q"], np.float32)
    W_K = np.asarray(inputs["W_K"], np.float32)
    W_V = np.asarray(inputs["W_V"], np.float32)
    W_O = np.asarray(inputs["W_O"], np.float32)
    if "nc" not in _cache:
        _cache["nc"] = build()
    nc = _cache["nc"]
    in_maps = make_in_maps(input_Q, input_K, input_V, W_Q, W_K, W_V, W_O)
    res = run_bass_kernel_spmd(nc, in_maps, list(range(N_CORES))).results

    def unblock(a):
        # [D*4, 512] blocks (J, m) -> [D, S]
        return a.reshape(4, 4, 128, 512).transpose(1, 2, 0, 3).reshape(D, S)

    out = np.empty((B, S, D), np.float32)
    for b in range(B):
        out[b] = unblock(res[2 * b]["outT"]).T.astype(np.float32) + unblock(
            res[2 * b + 1]["outT"]
        ).T.astype(np.float32)
    return out



# revision 76
# speedup vs baseline: 1.1967x; 1.1967x over previous
"""Causal multi-head attention (B=4, S=2048, D=512, H=8) on 8 trn2 cores.

Sharding: core c handles batch b = c//2 and head-group g = c%2 (4 heads).
Host pre-transposes activations into chunk-major contiguous blocks, casts
weights to bf16, and sums the two head-group partial outputs per batch
(the W_O row-parallel reduce).

Device kernel (per core), matmuls bf16 with f32 PSUM accumulation:
  QT/KT = W.T-slices @ x.T          [d=256, S]   (d on partitions)
  V     = x @ Wv.T-slice            packed as [V_h(64) | ones(64)] blocks
  ST    = K_h^T.T @ Q_h^T           [k, q] per 128-k-tile, block-causal,
                                    head pair on PE row groups 0:64/64:128
  P     = exp(ST/8)   split between ACT (spline exp) and DVE (Schraudolph
                      bitcast exp) in a DDAA period-4 pattern so the
                      st-psum rotation chain alternates engines
  pv    = [V_h|1].T @ P             M=128 -> [ctx_h(64); den_h(64)] per head
  rec   = exp(-ln(den)) on ACT (ln/exp share one table set)
  ctx   = pv_ctx * rec              DVE
  out.T = Wo-slice.T @ ctx          [512, S] bf16 partial (host adds pairs)

Emission is software-pipelined: PV for tile i-CARRY_LAG issues after the
ST/exp of tile i so the PE never stalls on one tile's exp; projections
and out-projections are interleaved between attention blocks as PE
filler; dummy matmuls keep the PE HAM clock-gate warm during the
DMA-bound head of the kernel.
"""
import sys

sys.path.insert(0, "/opt/trn_rl_repo")
from contextlib import ExitStack

import numpy as np
import ml_dtypes

import bass_rust
import concourse.bass as bass
import concourse.tile as tile
from concourse import mybir
from concourse.bass_utils import run_bass_kernel_spmd
from concourse.vector_clock import ScopedClock

BF16 = mybir.dt.bfloat16
F32 = mybir.dt.float32
I32 = mybir.dt.int32
I16 = mybir.dt.int16
F8 = mybir.dt.float8e4
DR = mybir.MatmulPerfMode.DoubleRow
EXP = mybir.ActivationFunctionType.Exp
LOG = mybir.ActivationFunctionType.Ln
MUL = mybir.AluOpType.mult
ADD = mybir.AluOpType.add

B, S, D, H = 4, 2048, 512, 8
# Schraudolph fast-exp on DVE: bf16bits(exp(x/8)) ~ i16((x*SCH_A + SCH_B)/65536)
SCH_A = 0.125 * (2**23) / float(np.log(2.0))
SCH_B = 1064987000.0
DK = 64          # head dim
HG = 2           # head groups (cores per batch)
NF = 4           # 128-rows tiles of the contraction dim D
NK = 16          # 128-wide k tiles
NJ = 4           # 512-wide q blocks
N_CORES = 8
WAITS_WIDE = 1

# knobs
EXP_DVE_OF_8 = 5   # of every 8 score tiles, this many exp on DVE
PG_BUFS = 7
CARRY_LAG = 5
ST_BUFS = 2
PV_BUFS = 4
# fp8e4m3 Q/K + DoubleRow perf mode for the score matmuls: measured on HW
# this gives NO matmul speedup (683ns vs 386ns bf16 for the same tile) and
# rel err 2.4e-2 > 2e-2 gate — keep disabled
ST_FP8 = False

# ---------------------------------------------------------------------------
# Workarounds for this walrus build: at most ONE sync wait per instruction.
_ctr = [0]


class _TC(tile.TileContext):
    def _drain_and_barrier(self, tick_clock, wait_clock):
        nc = self.nc
        drain_inst = nc.sync.drain()
        wait_clock.add_sem_waits(
            drain_inst.ins, ScopedClock({None: tick_clock.global_clock})
        )
        si = drain_inst.ins.sync_info
        waits = list(si.on_wait) if si is not None else []
        if waits:
            drain_inst.ins.sync_info = bass_rust.SyncInfo(
                on_wait=[], on_update=list(si.on_update)
            )
            for w in waits:
                nop = nc.sync.nop(nofuse=True)
                nop.ins.sync_info = bass_rust.SyncInfo(on_wait=[w], on_update=[])
        nc.all_engine_barrier()
        assert self.sems is not None
        popped = nc._tile_sem_poison_stack.pop()
        assert popped is self._sem_poison
        nc.clear_and_free_semaphores(list(self.sems.allocated().values()))
        # no trailing all_engine_barrier: the gpsimd sem clears are the last
        # instructions and the runtime waits for every engine queue to drain
        # before reporting completion, so the barrier only adds tail latency


def _fix_sync_waits(nc, maxw=1):
    for f in nc.m.functions:
        for bb in f.blocks:
            insts = list(bb.instructions)
            out = []
            dirty = False
            for inst in insts:
                si = inst.sync_info
                if si is not None:
                    waits = list(si.on_wait)
                    if isinstance(inst, mybir.InstDrain):
                        limit = 0
                    elif isinstance(inst, (mybir.InstMatmult, mybir.InstLdweights, mybir.InstActivation, mybir.InstTensorScalarPtr, mybir.InstTensorTensor)):
                        limit = WAITS_WIDE
                    else:
                        limit = maxw
                    if len(waits) > limit:
                        keep, extra = waits[:limit], waits[limit:]
                        for i in range(0, len(extra), maxw):
                            _ctr[0] += 1
                            nop = mybir.InstNoOp(name=f"ws-{_ctr[0]}")
                            nop.engine = inst.engine
                            nop.sync_info = bass_rust.SyncInfo(
                                on_wait=extra[i : i + maxw], on_update=[]
                            )
                            out.append(nop)
                        inst.sync_info = bass_rust.SyncInfo(
                            on_wait=keep, on_update=list(si.on_update)
                        )
                        dirty = True
                out.append(inst)
            if dirty:
                bb.instructions = out


# ---------------------------------------------------------------------------
def _make_pools(ctx, tc):
    return dict(
        cpool=ctx.enter_context(tc.tile_pool(name="const", bufs=1)),
        stp=ctx.enter_context(tc.tile_pool(name="stp", bufs=2 * ST_BUFS, space="PSUM")),
        # pv2 accumulators and the proj/out psums live in separate 1-slot
        # pools (2 banks each): the proj stream is woven into the attention
        # blocks as fillers, so its serialized evacuations hide behind
        # attention tiles instead of head-blocking the next block's exps
        pvp=ctx.enter_context(tc.tile_pool(name="pvp", bufs=1, space="PSUM")),
        projp=ctx.enter_context(tc.tile_pool(name="projp", bufs=1, space="PSUM")),
        ppool=ctx.enter_context(tc.tile_pool(name="ppool", bufs=PG_BUFS)),
        rpool=ctx.enter_context(tc.tile_pool(name="rpool", bufs=2)),
        opool=ctx.enter_context(tc.tile_pool(name="opool", bufs=2)),
        qpool=ctx.enter_context(tc.tile_pool(name="qpool", bufs=2)),
    )


def _emit_body(nc, tc, aps, pools, tag=""):
    xq, xk, xv, wq, wk, wv, wo, tri, outT = aps
    cpool = pools["cpool"]
    stp = pools["stp"]
    pvp = pools["pvp"]
    projp = pools["projp"]
    ppool = pools["ppool"]
    rpool = pools["rpool"]
    opool = pools["opool"]

    def ctile(shape, dtype, t):
        return cpool.tile(shape, dtype, tag=t + tag, name=t + tag)

    # ---- const tiles (f-merged so each input needs one DMA per chunk:
    # the sync engine pays ~600ns of descriptor issue per dma_start)
    wq_a = ctile([128, 1024], BF16, "wqa")   # f block at cols 256f
    wk_a = ctile([128, 1024], BF16, "wka")
    wv_a = ctile([128, 1024], BF16, "wva")
    xq_a = ctile([128, 4 * S], BF16, "xqa")  # f block at cols 2048f
    xk_a = ctile([128, 4 * S], BF16, "xka")
    xv_a = ctile([128, 4 * S], BF16, "xva")
    wo_a = ctile([128, 1024], BF16, "woa")  # t2 block at cols 512t2
    tri_t = ctile([128, 128], BF16, "tri")
    ones_t = ctile([128, 64], BF16, "ones")
    if ST_FP8:
        # per head h: a [32, (qk:2)x(i:2)x(s:S)] fp8 block, i = high/low
        # half of the 64-dim head (DoubleRow fold).  Matmul operand APs
        # only allow partition bases {0,32,64}, so head 3 shares base 64
        # with head 2 in a disjoint column range.
        QK8_PB = (0, 32, 64, 64)
        QK8_CB = (0, 0, 0, 4 * S)
        qk8 = ctile([128, 8 * S], F8, "qk8")
        qk_t = None
    else:
        # merged Q^T/K^T per head pair: qt at cols 0:S, kt at S:2S
        qk_t = [ctile([128, 2 * S], BF16, f"QK{t2}") for t2 in range(2)]
    # vhat: per k-tile i a [128, 512] block of 4 heads x [V_h(64) | ones(64)]
    vhat = ctile([128, NK * 512], BF16, "vhat")
    # merged ctx per J: head-pair t2 block at cols 512*t2
    ctx_j = [ctile([128, 1024], BF16, f"ctx{J}") for J in range(NJ)]

    def wslice(t, f, lo, hi):
        return t[:, 256 * f + lo : 256 * f + hi]

    def xslice(t, c, f, lo, hi):
        # x tiles are chunk-major: chunk c at cols 2048c, f-block at 512f
        return t[:, 2048 * c + 512 * f + lo : 2048 * c + 512 * f + hi]

    # ---- PE warmup: dummy matmuls on a memset tile spin the HAM clock gate
    # to K=8/8 while the input DMAs stream in.
    wup = ctile([128, 512], BF16, "wup")
    nc.gpsimd.memset(ones_t[:], 1.0)
    nc.gpsimd.memset(wup[:], 0.0)
    # ones blocks of vhat (cols 64:128 of each 128-block), set once
    nc.gpsimd.memset(
        vhat[:].rearrange("p (b w) -> p b w", w=128)[:, :, 64:128], 1.0
    )
    wps = stp.tile([128, 512], F32, tag="st", name=f"wps{tag}")
    for _ in range(12):
        nc.tensor.matmul(wps[:], wup[:, 0:128], wup[:], start=True, stop=True)

    # ---- DMA emission in consumption order.
    # Host arrays are pre-laid per-partition-contiguous (rows = partitions,
    # cols = the full sbuf free range), so every DMA is a plain 2D copy
    # with 2-4KB descriptors instead of 512B-1KB ones.
    def dma_w(t, ap, f, eng=None):
        (eng or nc.sync).dma_start(t[:], ap)

    def dma_x_chunk(t, ap, c, eng, split=1):
        for s in range(split):
            w = 2048 // split
            eng.dma_start(
                t[:, 2048 * c + w * s : 2048 * c + w * (s + 1)],
                ap[128 * c : 128 * (c + 1), w * s : w * (s + 1)],
            )

    # first-chunk DMAs issued from two engines in parallel so the critical
    # wq+xq0+wk+xk0 set is all in flight sooner
    dma_w(wq_a, wq, 4)
    dma_x_chunk(xq_a, xq, 0, nc.sync)
    dma_w(wk_a, wk, 4, nc.scalar)
    dma_x_chunk(xk_a, xk, 0, nc.scalar)
    nc.scalar.dma_start(tri_t[:], tri[:])
    dma_w(wv_a, wv, 4)
    dma_x_chunk(xv_a, xv, 0, nc.sync)
    for c in range(1, 4):
        dma_x_chunk(xq_a, xq, c, nc.sync)
        dma_x_chunk(xk_a, xk, c, nc.sync)
        if c == 1:
            dma_w(wo_a, wo, 2)
        dma_x_chunk(xv_a, xv, c, nc.sync)

    # ---- emit helpers -----------------------------------------------------
    # proj / V / out-proj emissions are chunked into closures ("fillers")
    # that the schedule weaves between attention tiles, so their psum
    # evacuations interleave with exps in the ACT/DVE queues instead of
    # head-blocking a block's first exps.
    def qk_fillers(t2, J):
        box = {}

        def mm_half(half, w_a, x_a):
            def f():
                if "ps" not in box:
                    box["ps"] = projp.tile(
                        [128, 1024], F32, tag="pj", name=f"psqk{t2}_{J}{tag}"
                    )
                    if J == 0:
                        # PE/HAM warmup while the input DMAs stream
                        for _ in range(3):
                            nc.tensor.matmul(
                                box["ps"][:, 0:512], wup[:, 0:128], wup[:],
                                start=True, stop=True,
                            )
                ps = box["ps"]
                for ff in range(NF):
                    nc.tensor.matmul(
                        ps[:, 512 * half : 512 * (half + 1)],
                        wslice(w_a, ff, 128 * t2, 128 * (t2 + 1)),
                        xslice(x_a, J, ff, 0, 512),
                        start=(ff == 0),
                        stop=(ff == NF - 1),
                    )
            return f

        def copy_q():
            ps = box["ps"]
            dst = qk_t[t2][:].rearrange("p (h s) -> p h s", h=2)[:, :, 512 * J : 512 * (J + 1)]
            nc.scalar.copy(dst[:, 0:1, :], ps[:, 0:512].rearrange("p (h w) -> p h w", h=1))

        def copy_k():
            ps = box["ps"]
            dst = qk_t[t2][:].rearrange("p (h s) -> p h s", h=2)[:, :, 512 * J : 512 * (J + 1)]
            nc.vector.tensor_copy(dst[:, 1:2, :], ps[:, 512:1024].rearrange("p (h w) -> p h w", h=1))

        return [mm_half(0, wq_a, xq_a), mm_half(1, wk_a, xk_a), copy_q, copy_k]

    def v_fillers(c):
        # V for k-tiles i in [4c, 4c+4): one [128,1024] psum (4 x 256),
        # strided casts into the V-halves of vhat's [V|1] blocks
        box = {}

        def mm_pair(lo):
            def f():
                if "ps" not in box:
                    box["ps"] = projp.tile([128, 1024], F32, tag="pj", name=f"psv{c}{tag}")
                    if c == 0:
                        for _ in range(3):
                            nc.tensor.matmul(
                                box["ps"][:, 0:512], wup[:, 0:128], wup[:],
                                start=True, stop=True,
                            )
                ps = box["ps"]
                for ii in range(lo, lo + 2):
                    for ff in range(NF):
                        nc.tensor.matmul(
                            ps[:, 256 * ii : 256 * (ii + 1)],
                            xslice(xv_a, c, ff, 128 * ii, 128 * (ii + 1)),
                            wslice(wv_a, ff, 0, 256),
                            start=(ff == 0),
                            stop=(ff == NF - 1),
                        )
            return f

        def cast_lo():
            dst = vhat[:, 2048 * c : 2048 * (c + 1)].rearrange("p (b w) -> p b w", w=128)
            nc.vector.tensor_copy(
                dst[:, 0:8, 0:64], box["ps"][:, 0:512].rearrange("p (b w) -> p b w", w=64)
            )

        def cast_hi():
            dst = vhat[:, 2048 * c : 2048 * (c + 1)].rearrange("p (b w) -> p b w", w=128)
            nc.scalar.copy(
                dst[:, 8:16, 0:64], box["ps"][:, 512:1024].rearrange("p (b w) -> p b w", w=64)
            )

        return [mm_pair(0), mm_pair(2), cast_lo, cast_hi]

    ec = [0]
    carry = []  # global pending PV-flush closures (pipeline depth 2)

    def step_carry():
        carry.pop(0)()

    def drain_carry():
        while carry:
            step_carry()

    def emit_att(J, hp, fill=None):
        fill = fill if fill is not None else []
        nk = 4 * (J + 1)
        ha, hb = 2 * hp, 2 * hp + 1
        qk = None if ST_FP8 else qk_t[hp]

        def st_mm(h, out_ap, i, qlo):
            pb, cb = QK8_PB[h], QK8_CB[h]
            t = qk8[pb : pb + 32, cb : cb + 4 * S].rearrange("p (qk i s) -> p qk i s", qk=2, i=2)
            nc.tensor.matmul(
                out_ap,
                t[:, 1, :, 128 * i : 128 * (i + 1)],
                t[:, 0, :, 512 * J + qlo : 512 * (J + 1)],
                start=True, stop=True, perf_mode=DR,
            )
        # pv2: head a at cols 0:512, head b at 512:1024; each [ctx(64); den(64)]
        pv2 = pvp.tile([128, 1024], F32, tag="pv", name=f"pv{J}_{hp}{tag}")

        def emit_recip():
            # Copy pv2 out to SBUF immediately (releases the PSUM bank pair
            # for the next block), then 1/den = exp(-log(den)) on ACT and
            # per head ctx = pv_ctx * rec on DVE from the copy.  The final
            # block is latency-critical (nothing left to overlap), so its
            # two heads run on ACT and DVE concurrently.
            lg = rpool.tile([64, 1024], F32, tag="rect", name=f"rect{J}_{hp}{tag}")
            rec = rpool.tile([64, 1024], F32, tag="rec", name=f"rec{J}_{hp}{tag}")
            cj = ctx_j[J][:, 512 * hp : 512 * (hp + 1)]
            if (J == NJ - 1) and (hp == 1):
                # tail: read psum directly, heads split ACT / DVE-Newton
                den = pv2[64:128, :]
                nc.scalar.activation(lg[:, 0:512], den[:, 0:512], LOG)
                nc.scalar.activation(rec[:, 0:512], lg[:, 0:512], EXP, scale=-1.0)
                nc.vector.tensor_scalar(
                    rec[:, 512:1024].bitcast(I32), den[:, 512:1024].bitcast(I32),
                    -1, 0x7EF311C3, MUL, ADD,
                )
                nc.vector.tensor_mul(lg[:, 512:1024], den[:, 512:1024], rec[:, 512:1024])
                nc.vector.scalar_tensor_tensor(
                    lg[:, 512:1024], lg[:, 512:1024], 2.0, rec[:, 512:1024],
                    mybir.AluOpType.subtract, MUL,
                )
                nc.vector.tensor_mul(cj[0:64, :], pv2[0:64, 0:512], rec[:, 0:512])
                nc.vector.scalar_tensor_tensor(
                    cj[64:128, :], pv2[0:64, 512:1024], -1.0, lg[:, 512:1024],
                    MUL, MUL,
                )
            else:
                nc.scalar.activation(lg[:], pv2[64:128, :], LOG)
                nc.scalar.activation(rec[:], lg[:], EXP, scale=-1.0)
                nc.vector.tensor_mul(cj[0:64, :], pv2[0:64, 0:512], rec[:, 0:512])
                nc.vector.tensor_mul(cj[64:128, :], pv2[0:64, 512:1024], rec[:, 512:1024])

        def make_flush(i, pg, qlo, st_, sp_):
            def fl():
                va = vhat[:, 512 * i + 128 * ha : 512 * i + 128 * (ha + 1)]
                vb = vhat[:, 512 * i + 128 * hb : 512 * i + 128 * (hb + 1)]
                nc.tensor.matmul(
                    pv2[:, qlo:512], va, pg[:, qlo:512], start=st_, stop=sp_,
                )
                nc.tensor.matmul(
                    pv2[:, 512 + qlo : 1024], vb, pg[:, 512 + qlo : 1024],
                    start=st_, stop=sp_,
                )
                if sp_:
                    emit_recip()
            return fl

        # tile 0 first (full-width, unmasked: the PSUM-start flush never
        # waits on the gpsimd mask chain), then diagonal (masked) tiles so
        # their exp->mask chain gets CARRY_LAG tiles of slack, then the
        # remaining off-diagonal tiles so the block-end carry drain flushes
        # tiles whose pg needs no gpsimd hop.
        if J == 0:
            order = [0, 1, 2, 3]
        else:
            order = [0] + list(range(4 * J, nk)) + list(range(1, 4 * J))
        for idx, i in enumerate(order):
            dq = i - 4 * J
            qlo = 128 * dq if dq > 0 else 0
            # per-head score tiles: halves the exp latency in the stp ring
            # (DVE exps head a, ACT exps head b, concurrently on every tile)
            sta = stp.tile([128, 512], F32, tag="st", name=f"sta{J}_{hp}_{i}{tag}")
            stb = stp.tile([128, 512], F32, tag="st", name=f"stb{J}_{hp}_{i}{tag}")
            pg = ppool.tile([128, 1024], BF16, tag="pg", name=f"pg{J}_{hp}_{i}{tag}")
            if ST_FP8:
                st_mm(ha, sta[:, qlo:512], i, qlo)
                st_mm(hb, stb[:, qlo:512], i, qlo)
            else:
                nc.tensor.matmul(
                    sta[:, qlo:512],
                    qk[0:64, S + 128 * i : S + 128 * (i + 1)],
                    qk[0:64, 512 * J + qlo : 512 * (J + 1)],
                    start=True, stop=True,
                )
                nc.tensor.matmul(
                    stb[:, qlo:512],
                    qk[64:128, S + 128 * i : S + 128 * (i + 1)],
                    qk[64:128, 512 * J + qlo : 512 * (J + 1)],
                    start=True, stop=True,
                )
            nc.vector.tensor_scalar(
                pg[:, qlo:512].bitcast(I16), sta[:, qlo:512],
                SCH_A / 65536.0, SCH_B / 65536.0, MUL, ADD,
            )
            if idx <= 1:
                # the previous block's reciprocal (~1.8us) heads the ACT
                # queue at block start; the first two tiles' head-b exps go
                # to DVE so the block's first PV flushes never wait on that
                # backlog
                nc.vector.tensor_scalar(
                    pg[:, 512 + qlo : 1024].bitcast(I16), stb[:, qlo:512],
                    SCH_A / 65536.0, SCH_B / 65536.0, MUL, ADD,
                )
            else:
                nc.scalar.activation(
                    pg[:, 512 + qlo : 1024], stb[:, qlo:512], EXP, scale=0.125
                )
            if dq >= 0:
                nc.gpsimd.tensor_mul(pg[:, qlo : qlo + 128], pg[:, qlo : qlo + 128], tri_t[:])
                nc.gpsimd.tensor_mul(
                    pg[:, 512 + qlo : 512 + qlo + 128],
                    pg[:, 512 + qlo : 512 + qlo + 128], tri_t[:],
                )
            if len(carry) >= CARRY_LAG:
                step_carry()
            carry.append(make_flush(i, pg, qlo, idx == 0, idx == nk - 1))
            # weave in pending proj/out work between attention tiles; short
            # blocks pop faster so fewer fillers drain in a burst at block
            # end (right where the next block's exps queue)
            if idx >= 1:
                for _ in range(2 if nk > 8 else 3):
                    if fill:
                        fill.pop(0)()
        drain_carry()
        # anything not absorbed must still be emitted before the next block
        # (the dependency tracker keys on emission order)
        while fill:
            fill.pop(0)()

    def po_mm(po, J, mp):
        for mh in range(2):
            m = 2 * mp + mh
            for t2 in range(2):
                nc.tensor.matmul(
                    po[:, 512 * mh : 512 * (mh + 1)],
                    wo_a[:, 512 * t2 + 128 * m : 512 * t2 + 128 * (m + 1)],
                    ctx_j[J][:, 512 * t2 : 512 * (t2 + 1)],
                    start=(t2 == 0), stop=(t2 == 1),
                )

    def out_fillers(J):
        box = {}
        out3 = outT[512 * J : 512 * (J + 1), :].rearrange("(m p) w -> p m w", m=4)

        def mm(mp):
            def f():
                if "ob" not in box:
                    box["ob"] = opool.tile([128, 2048], BF16, tag="ob", name=f"ob{J}{tag}")
                box[mp] = projp.tile([128, 1024], F32, tag="pj", name=f"po{J}_{mp}{tag}")
                po_mm(box[mp], J, mp)
            return f

        def cp(mp):
            def f():
                ob = box["ob"]
                if mp == 0:
                    nc.scalar.copy(ob[:, 0:1024], box[0][:])
                else:
                    nc.vector.tensor_copy(ob[:, 1024:2048], box[1][:])
            return f

        def dma(mp):
            def f():
                (nc.scalar if mp == 0 else nc.sync).dma_start(
                    out3[:, 2 * mp : 2 * mp + 2, :],
                    box["ob"][:, 1024 * mp : 1024 * (mp + 1)].rearrange("p (m w) -> p m w", m=2),
                )
            return f

        return [mm(0), cp(0), dma(0), mm(1), cp(1), dma(1)]

    def emit_out_last(J):
        # tail is latency-critical: copy halves on both engines in
        # parallel and fire each half's DMA as soon as it lands
        ob = opool.tile([128, 2048], BF16, tag="ob", name=f"ob{J}{tag}")
        out3 = outT[512 * J : 512 * (J + 1), :].rearrange("(m p) w -> p m w", m=4)
        for mp in range(2):
            # mp=1 reuses pv2's pool: its slot frees exactly when the final
            # recip (the tail dependency anyway) finishes reading pv2
            pool = projp if mp == 0 else pvp
            po = pool.tile([128, 1024], F32, tag="pj" if mp == 0 else "pv", name=f"po{J}_{mp}{tag}")
            po_mm(po, J, mp)
            for mh in range(2):
                m = 2 * mp + mh
                cp_ = nc.scalar.copy if mh == 0 else nc.vector.tensor_copy
                cp_(
                    ob[:, 1024 * mp + 512 * mh : 1024 * mp + 512 * (mh + 1)],
                    po[:, 512 * mh : 512 * (mh + 1)],
                )
                (nc.scalar if mh else nc.sync).dma_start(
                    out3[:, m : m + 1, :],
                    ob[:, 1024 * mp + 512 * mh : 1024 * mp + 512 * (mh + 1)].rearrange(
                        "p (m w) -> p m w", m=1
                    ),
                )

    # ---- schedule: J=0 projections run up front (DMA-bound phase); all
    # later proj/V/out work is woven between attention tiles as fillers.
    for f in qk_fillers(0, 0) + qk_fillers(1, 0) + v_fillers(0):
        f()
    fill = []
    for J in range(NJ):
        if J > 0:
            fill += out_fillers(J - 1)
        emit_att(J, 0, fill)
        if J + 1 < NJ:
            fill += v_fillers(J + 1)
            fill += qk_fillers(0, J + 1)
            fill += qk_fillers(1, J + 1)
        emit_att(J, 1, fill)
    # dummy matmuls bridge the final-reciprocal PE gap so the HAM clock
    # gate stays at full rate for the last out-projection matmuls
    wfin = stp.tile([128, 512], F32, tag="st", name=f"wfin{tag}")
    for _ in range(12):
        nc.tensor.matmul(wfin[:], wup[:, 0:128], wup[:], start=True, stop=True)
    emit_out_last(NJ - 1)


def build(repeat=1, dup=frozenset()):
    nc = bass.Bass("TRN2", target_bir_lowering=False, debug=False, num_devices=N_CORES)
    xq = nc.dram_tensor("xq", [512, 2048], BF16, kind="ExternalInput").ap()
    xk = nc.dram_tensor("xk", [512, 2048], BF16, kind="ExternalInput").ap()
    xv = nc.dram_tensor("xv", [512, 2048], BF16, kind="ExternalInput").ap()
    wq = nc.dram_tensor("wq", [128, 1024], BF16, kind="ExternalInput").ap()
    wk = nc.dram_tensor("wk", [128, 1024], BF16, kind="ExternalInput").ap()
    wv = nc.dram_tensor("wv", [128, 1024], BF16, kind="ExternalInput").ap()
    wo = nc.dram_tensor("wo", [128, 1024], BF16, kind="ExternalInput").ap()
    tri = nc.dram_tensor("tri", [128, 128], BF16, kind="ExternalInput").ap()
    outT = nc.dram_tensor("outT", [D * 4, 512], BF16, kind="ExternalOutput").ap()
    aps = (xq, xk, xv, wq, wk, wv, wo, tri, outT)
    with _TC(nc) as tc:
        with ExitStack() as ctx:
            pools = _make_pools(ctx, tc)
            for r in range(repeat):
                _emit_body(nc, tc, aps, pools, tag=(f"_r{r}" if r else ""))
    _fix_sync_waits(nc)
    return nc


def make_in_maps(input_Q, input_K, input_V, W_Q, W_K, W_V, W_O):
    bf = ml_dtypes.bfloat16

    def chunked(xTb):
        # [D, S] -> [4c*128p, (f w)]: chunk c at rows 128c, each partition
        # row holds its full 4KB of (f, w) data contiguously
        a = xTb.reshape(4, 128, 4, 512).transpose(2, 1, 0, 3)
        return np.ascontiguousarray(a.reshape(512, 2048))

    def wfold(wT, f):
        # [f*128, w] -> [128, f*w]: per-partition f-major contiguous
        a = wT.reshape(f, 128, -1).transpose(1, 0, 2)
        return np.ascontiguousarray(a.reshape(128, -1))

    xT = {}
    for b in range(B):
        xT[("q", b)] = chunked(input_Q[b].T.astype(bf))
        xT[("k", b)] = chunked(input_K[b].T.astype(bf))
        xT[("v", b)] = chunked(input_V[b].T.astype(bf))
    wslices = {}
    for g in range(HG):
        rows = slice(256 * g, 256 * (g + 1))
        wslices[("wq", g)] = wfold(np.ascontiguousarray(W_Q[rows, :].T).astype(bf), 4)
        wslices[("wk", g)] = wfold(np.ascontiguousarray(W_K[rows, :].T).astype(bf), 4)
        wslices[("wv", g)] = wfold(np.ascontiguousarray(W_V[rows, :].T).astype(bf), 4)
        wslices[("wo", g)] = wfold(np.ascontiguousarray(W_O[:, rows].T).astype(bf), 2)
    kk, qq = np.meshgrid(np.arange(128), np.arange(128), indexing="ij")
    tri_np = (qq >= kk).astype(bf)
    in_maps = []
    for c in range(N_CORES):
        b, g = c // HG, c % HG
        in_maps.append(
            {
                "xq": xT[("q", b)],
                "xk": xT[("k", b)],
                "xv": xT[("v", b)],
                "wq": wslices[("wq", g)],
                "wk": wslices[("wk", g)],
                "wv": wslices[("wv", g)],
                "wo": wslices[("wo", g)],
                "tri": tri_np,
            }
        )
    return in_maps


_cache = {}


def kernel(**inputs):
    input_Q = np.asarray(inputs["input_Q"], np.float32)
    input_K = np.asarray(inputs["input_K"], np.float32)
    input_V = np.asarray(inputs["input_V"], np.float32)
    W_Q = np.asarray(inputs["W_Q"], np.float32)
    W_K = np.asarray(inputs["W_K"], np.float32)
    W_V = np.asarray(inputs["W_V"], np.float32)
    W_O = np.asarray(inputs["W_O"], np.float32)
    if "nc" not in _cache:
        _cache["nc"] = build()
    nc = _cache["nc"]
    in_maps = make_in_maps(input_Q, input_K, input_V, W_Q, W_K, W_V, W_O)
    res = run_bass_kernel_spmd(nc, in_maps, list(range(N_CORES))).results

    def unblock(a):
        # [D*4, 512] blocks (J, m) -> [D, S]
        return a.reshape(4, 4, 128, 512).transpose(1, 2, 0, 3).reshape(D, S)

    out = np.empty((B, S, D), np.float32)
    for b in range(B):
        out[b] = unblock(res[2 * b]["outT"]).T.astype(np.float32) + unblock(
            res[2 * b + 1]["outT"]
        ).T.astype(np.float32)
    return out



# revision 78
# speedup vs baseline: 1.1998x; 1.0026x over previous
"""Causal multi-head attention (B=4, S=2048, D=512, H=8) on 8 trn2 cores.

Sharding: core c handles batch b = c//2 and head-group g = c%2 (4 heads).
Host pre-transposes activations into chunk-major contiguous blocks, casts
weights to bf16, and sums the two head-group partial outputs per batch
(the W_O row-parallel reduce).

Device kernel (per core), matmuls bf16 with f32 PSUM accumulation:
  QT/KT = W.T-slices @ x.T          [d=256, S]   (d on partitions)
  V     = x @ Wv.T-slice            packed as [V_h(64) | ones(64)] blocks
  ST    = K_h^T.T @ Q_h^T           [k, q] per 128-k-tile, block-causal,
                                    head pair on PE row groups 0:64/64:128
  P     = exp(ST/8)   split between ACT (spline exp) and DVE (Schraudolph
                      bitcast exp) in a DDAA period-4 pattern so the
                      st-psum rotation chain alternates engines
  pv    = [V_h|1].T @ P             M=128 -> [ctx_h(64); den_h(64)] per head
  rec   = exp(-ln(den)) on ACT (ln/exp share one table set)
  ctx   = pv_ctx * rec              DVE
  out.T = Wo-slice.T @ ctx          [512, S] bf16 partial (host adds pairs)

Emission is software-pipelined: PV for tile i-CARRY_LAG issues after the
ST/exp of tile i so the PE never stalls on one tile's exp; projections
and out-projections are interleaved between attention blocks as PE
filler; dummy matmuls keep the PE HAM clock-gate warm during the
DMA-bound head of the kernel.
"""
import sys

sys.path.insert(0, "/opt/trn_rl_repo")
from contextlib import ExitStack

import numpy as np
import ml_dtypes

import bass_rust
import concourse.bass as bass
import concourse.tile as tile
from concourse import mybir
from concourse.bass_utils import run_bass_kernel_spmd
from concourse.vector_clock import ScopedClock

BF16 = mybir.dt.bfloat16
F32 = mybir.dt.float32
I32 = mybir.dt.int32
I16 = mybir.dt.int16
F8 = mybir.dt.float8e4
DR = mybir.MatmulPerfMode.DoubleRow
EXP = mybir.ActivationFunctionType.Exp
LOG = mybir.ActivationFunctionType.Ln
MUL = mybir.AluOpType.mult
ADD = mybir.AluOpType.add

B, S, D, H = 4, 2048, 512, 8
# Schraudolph fast-exp on DVE: bf16bits(exp(x/8)) ~ i16((x*SCH_A + SCH_B)/65536)
SCH_A = 0.125 * (2**23) / float(np.log(2.0))
SCH_B = 1064987000.0
DK = 64          # head dim
HG = 2           # head groups (cores per batch)
NF = 4           # 128-rows tiles of the contraction dim D
NK = 16          # 128-wide k tiles
NJ = 4           # 512-wide q blocks
N_CORES = 8
WAITS_WIDE = 1

# knobs
EXP_DVE_OF_8 = 5   # of every 8 score tiles, this many exp on DVE
PG_BUFS = 7
CARRY_LAG = 5
ST_BUFS = 2
PV_BUFS = 4
# fp8e4m3 Q/K + DoubleRow perf mode for the score matmuls: measured on HW
# this gives NO matmul speedup (683ns vs 386ns bf16 for the same tile) and
# rel err 2.4e-2 > 2e-2 gate — keep disabled
ST_FP8 = False

# ---------------------------------------------------------------------------
# Workarounds for this walrus build: at most ONE sync wait per instruction.
_ctr = [0]


class _TC(tile.TileContext):
    def _drain_and_barrier(self, tick_clock, wait_clock):
        nc = self.nc
        drain_inst = nc.sync.drain()
        wait_clock.add_sem_waits(
            drain_inst.ins, ScopedClock({None: tick_clock.global_clock})
        )
        si = drain_inst.ins.sync_info
        waits = list(si.on_wait) if si is not None else []
        if waits:
            drain_inst.ins.sync_info = bass_rust.SyncInfo(
                on_wait=[], on_update=list(si.on_update)
            )
            for w in waits:
                nop = nc.sync.nop(nofuse=True)
                nop.ins.sync_info = bass_rust.SyncInfo(on_wait=[w], on_update=[])
        nc.all_engine_barrier()
        assert self.sems is not None
        popped = nc._tile_sem_poison_stack.pop()
        assert popped is self._sem_poison
        nc.clear_and_free_semaphores(list(self.sems.allocated().values()))
        # no trailing all_engine_barrier: the gpsimd sem clears are the last
        # instructions and the runtime waits for every engine queue to drain
        # before reporting completion, so the barrier only adds tail latency


def _fix_sync_waits(nc, maxw=1):
    for f in nc.m.functions:
        for bb in f.blocks:
            insts = list(bb.instructions)
            out = []
            dirty = False
            for inst in insts:
                si = inst.sync_info
                if si is not None:
                    waits = list(si.on_wait)
                    if isinstance(inst, mybir.InstDrain):
                        limit = 0
                    elif isinstance(inst, (mybir.InstMatmult, mybir.InstLdweights, mybir.InstActivation, mybir.InstTensorScalarPtr, mybir.InstTensorTensor)):
                        limit = WAITS_WIDE
                    else:
                        limit = maxw
                    if len(waits) > limit:
                        keep, extra = waits[:limit], waits[limit:]
                        for i in range(0, len(extra), maxw):
                            _ctr[0] += 1
                            nop = mybir.InstNoOp(name=f"ws-{_ctr[0]}")
                            nop.engine = inst.engine
                            nop.sync_info = bass_rust.SyncInfo(
                                on_wait=extra[i : i + maxw], on_update=[]
                            )
                            out.append(nop)
                        inst.sync_info = bass_rust.SyncInfo(
                            on_wait=keep, on_update=list(si.on_update)
                        )
                        dirty = True
                out.append(inst)
            if dirty:
                bb.instructions = out


# ---------------------------------------------------------------------------
def _make_pools(ctx, tc):
    return dict(
        cpool=ctx.enter_context(tc.tile_pool(name="const", bufs=1)),
        stp=ctx.enter_context(tc.tile_pool(name="stp", bufs=2 * ST_BUFS, space="PSUM")),
        # pv2 accumulators and the proj/out psums live in separate 1-slot
        # pools (2 banks each): the proj stream is woven into the attention
        # blocks as fillers, so its serialized evacuations hide behind
        # attention tiles instead of head-blocking the next block's exps
        pvp=ctx.enter_context(tc.tile_pool(name="pvp", bufs=1, space="PSUM")),
        projp=ctx.enter_context(tc.tile_pool(name="projp", bufs=1, space="PSUM")),
        ppool=ctx.enter_context(tc.tile_pool(name="ppool", bufs=PG_BUFS)),
        rpool=ctx.enter_context(tc.tile_pool(name="rpool", bufs=2)),
        opool=ctx.enter_context(tc.tile_pool(name="opool", bufs=2)),
        qpool=ctx.enter_context(tc.tile_pool(name="qpool", bufs=2)),
    )


def _emit_body(nc, tc, aps, pools, tag=""):
    xq, xk, xv, wq, wk, wv, wo, tri, outT = aps
    cpool = pools["cpool"]
    stp = pools["stp"]
    pvp = pools["pvp"]
    projp = pools["projp"]
    ppool = pools["ppool"]
    rpool = pools["rpool"]
    opool = pools["opool"]

    def ctile(shape, dtype, t):
        return cpool.tile(shape, dtype, tag=t + tag, name=t + tag)

    # ---- const tiles (f-merged so each input needs one DMA per chunk:
    # the sync engine pays ~600ns of descriptor issue per dma_start)
    wq_a = ctile([128, 1024], BF16, "wqa")   # f block at cols 256f
    wk_a = ctile([128, 1024], BF16, "wka")
    wv_a = ctile([128, 1024], BF16, "wva")
    xq_a = ctile([128, 4 * S], BF16, "xqa")  # f block at cols 2048f
    xk_a = ctile([128, 4 * S], BF16, "xka")
    xv_a = ctile([128, 4 * S], BF16, "xva")
    wo_a = ctile([128, 1024], BF16, "woa")  # t2 block at cols 512t2
    tri_t = ctile([128, 128], BF16, "tri")
    ones_t = ctile([128, 64], BF16, "ones")
    if ST_FP8:
        # per head h: a [32, (qk:2)x(i:2)x(s:S)] fp8 block, i = high/low
        # half of the 64-dim head (DoubleRow fold).  Matmul operand APs
        # only allow partition bases {0,32,64}, so head 3 shares base 64
        # with head 2 in a disjoint column range.
        QK8_PB = (0, 32, 64, 64)
        QK8_CB = (0, 0, 0, 4 * S)
        qk8 = ctile([128, 8 * S], F8, "qk8")
        qk_t = None
    else:
        # merged Q^T/K^T per head pair: qt at cols 0:S, kt at S:2S
        qk_t = [ctile([128, 2 * S], BF16, f"QK{t2}") for t2 in range(2)]
    # vhat: per k-tile i a [128, 512] block of 4 heads x [V_h(64) | ones(64)]
    vhat = ctile([128, NK * 512], BF16, "vhat")
    # merged ctx per J: head-pair t2 block at cols 512*t2
    ctx_j = [ctile([128, 1024], BF16, f"ctx{J}") for J in range(NJ)]

    def wslice(t, f, lo, hi):
        return t[:, 256 * f + lo : 256 * f + hi]

    def xslice(t, c, f, lo, hi):
        # x tiles are chunk-major: chunk c at cols 2048c, f-block at 512f
        return t[:, 2048 * c + 512 * f + lo : 2048 * c + 512 * f + hi]

    # ---- PE warmup: dummy matmuls on a memset tile spin the HAM clock gate
    # to K=8/8 while the input DMAs stream in.
    wup = ctile([128, 512], BF16, "wup")
    nc.gpsimd.memset(ones_t[:], 1.0)
    nc.gpsimd.memset(wup[:], 0.0)
    # ones blocks of vhat (cols 64:128 of each 128-block), set once
    nc.gpsimd.memset(
        vhat[:].rearrange("p (b w) -> p b w", w=128)[:, :, 64:128], 1.0
    )
    wps = stp.tile([128, 512], F32, tag="st", name=f"wps{tag}")
    for _ in range(12):
        nc.tensor.matmul(wps[:], wup[:, 0:128], wup[:], start=True, stop=True)

    # ---- DMA emission in consumption order.
    # Host arrays are pre-laid per-partition-contiguous (rows = partitions,
    # cols = the full sbuf free range), so every DMA is a plain 2D copy
    # with 2-4KB descriptors instead of 512B-1KB ones.
    def dma_w(t, ap, f, eng=None):
        (eng or nc.sync).dma_start(t[:], ap)

    def dma_x_chunk(t, ap, c, eng, split=1):
        for s in range(split):
            w = 2048 // split
            eng.dma_start(
                t[:, 2048 * c + w * s : 2048 * c + w * (s + 1)],
                ap[128 * c : 128 * (c + 1), w * s : w * (s + 1)],
            )

    # first-chunk DMAs issued from two engines in parallel so the critical
    # wq+xq0+wk+xk0 set is all in flight sooner
    dma_w(wq_a, wq, 4)
    dma_x_chunk(xq_a, xq, 0, nc.sync)
    dma_w(wk_a, wk, 4, nc.scalar)
    dma_x_chunk(xk_a, xk, 0, nc.scalar)
    nc.scalar.dma_start(tri_t[:], tri[:])
    dma_w(wv_a, wv, 4)
    dma_x_chunk(xv_a, xv, 0, nc.sync)
    for c in range(1, 4):
        dma_x_chunk(xq_a, xq, c, nc.sync)
        dma_x_chunk(xk_a, xk, c, nc.sync)
        if c == 1:
            dma_w(wo_a, wo, 2)
        dma_x_chunk(xv_a, xv, c, nc.sync)

    # ---- emit helpers -----------------------------------------------------
    # proj / V / out-proj emissions are chunked into closures ("fillers")
    # that the schedule weaves between attention tiles, so their psum
    # evacuations interleave with exps in the ACT/DVE queues instead of
    # head-blocking a block's first exps.
    def qk_fillers(t2, J):
        box = {}

        def mm_half(half, w_a, x_a):
            def f():
                if "ps" not in box:
                    box["ps"] = projp.tile(
                        [128, 1024], F32, tag="pj", name=f"psqk{t2}_{J}{tag}"
                    )
                    if J == 0:
                        # PE/HAM warmup while the input DMAs stream
                        for _ in range(3):
                            nc.tensor.matmul(
                                box["ps"][:, 0:512], wup[:, 0:128], wup[:],
                                start=True, stop=True,
                            )
                ps = box["ps"]
                for ff in range(NF):
                    nc.tensor.matmul(
                        ps[:, 512 * half : 512 * (half + 1)],
                        wslice(w_a, ff, 128 * t2, 128 * (t2 + 1)),
                        xslice(x_a, J, ff, 0, 512),
                        start=(ff == 0),
                        stop=(ff == NF - 1),
                    )
            return f

        def copy_q():
            ps = box["ps"]
            dst = qk_t[t2][:].rearrange("p (h s) -> p h s", h=2)[:, :, 512 * J : 512 * (J + 1)]
            nc.scalar.copy(dst[:, 0:1, :], ps[:, 0:512].rearrange("p (h w) -> p h w", h=1))

        def copy_k():
            ps = box["ps"]
            dst = qk_t[t2][:].rearrange("p (h s) -> p h s", h=2)[:, :, 512 * J : 512 * (J + 1)]
            nc.vector.tensor_copy(dst[:, 1:2, :], ps[:, 512:1024].rearrange("p (h w) -> p h w", h=1))

        return [mm_half(0, wq_a, xq_a), mm_half(1, wk_a, xk_a), copy_q, copy_k]

    def v_fillers(c):
        # V for k-tiles i in [4c, 4c+4): one [128,1024] psum (4 x 256),
        # strided casts into the V-halves of vhat's [V|1] blocks
        box = {}

        def mm_pair(lo):
            def f():
                if "ps" not in box:
                    box["ps"] = projp.tile([128, 1024], F32, tag="pj", name=f"psv{c}{tag}")
                    if c == 0:
                        for _ in range(3):
                            nc.tensor.matmul(
                                box["ps"][:, 0:512], wup[:, 0:128], wup[:],
                                start=True, stop=True,
                            )
                ps = box["ps"]
                for ii in range(lo, lo + 2):
                    for ff in range(NF):
                        nc.tensor.matmul(
                            ps[:, 256 * ii : 256 * (ii + 1)],
                            xslice(xv_a, c, ff, 128 * ii, 128 * (ii + 1)),
                            wslice(wv_a, ff, 0, 256),
                            start=(ff == 0),
                            stop=(ff == NF - 1),
                        )
            return f

        def cast_lo():
            dst = vhat[:, 2048 * c : 2048 * (c + 1)].rearrange("p (b w) -> p b w", w=128)
            nc.vector.tensor_copy(
                dst[:, 0:8, 0:64], box["ps"][:, 0:512].rearrange("p (b w) -> p b w", w=64)
            )

        def cast_hi():
            dst = vhat[:, 2048 * c : 2048 * (c + 1)].rearrange("p (b w) -> p b w", w=128)
            nc.scalar.copy(
                dst[:, 8:16, 0:64], box["ps"][:, 512:1024].rearrange("p (b w) -> p b w", w=64)
            )

        return [mm_pair(0), mm_pair(2), cast_lo, cast_hi]

    ec = [0]
    carry = []  # global pending PV-flush closures (pipeline depth 2)

    def step_carry():
        carry.pop(0)()

    def drain_carry():
        while carry:
            step_carry()

    def emit_att(J, hp, fill=None):
        fill = fill if fill is not None else []
        nk = 4 * (J + 1)
        ha, hb = 2 * hp, 2 * hp + 1
        qk = None if ST_FP8 else qk_t[hp]

        def st_mm(h, out_ap, i, qlo):
            pb, cb = QK8_PB[h], QK8_CB[h]
            t = qk8[pb : pb + 32, cb : cb + 4 * S].rearrange("p (qk i s) -> p qk i s", qk=2, i=2)
            nc.tensor.matmul(
                out_ap,
                t[:, 1, :, 128 * i : 128 * (i + 1)],
                t[:, 0, :, 512 * J + qlo : 512 * (J + 1)],
                start=True, stop=True, perf_mode=DR,
            )
        # pv2: head a at cols 0:512, head b at 512:1024; each [ctx(64); den(64)]
        pv2 = pvp.tile([128, 1024], F32, tag="pv", name=f"pv{J}_{hp}{tag}")

        def emit_recip():
            # Copy pv2 out to SBUF immediately (releases the PSUM bank pair
            # for the next block), then 1/den = exp(-log(den)) on ACT and
            # per head ctx = pv_ctx * rec on DVE from the copy.  The final
            # block is latency-critical (nothing left to overlap), so its
            # two heads run on ACT and DVE concurrently.
            lg = rpool.tile([64, 1024], F32, tag="rect", name=f"rect{J}_{hp}{tag}")
            rec = rpool.tile([64, 1024], F32, tag="rec", name=f"rec{J}_{hp}{tag}")
            cj = ctx_j[J][:, 512 * hp : 512 * (hp + 1)]
            if (J == NJ - 1) and (hp == 1):
                # tail: read psum directly, heads split ACT / DVE-Newton
                den = pv2[64:128, :]
                nc.scalar.activation(lg[:, 0:512], den[:, 0:512], LOG)
                nc.scalar.activation(rec[:, 0:512], lg[:, 0:512], EXP, scale=-1.0)
                nc.vector.tensor_scalar(
                    rec[:, 512:1024].bitcast(I32), den[:, 512:1024].bitcast(I32),
                    -1, 0x7EF311C3, MUL, ADD,
                )
                nc.vector.tensor_mul(lg[:, 512:1024], den[:, 512:1024], rec[:, 512:1024])
                nc.vector.scalar_tensor_tensor(
                    lg[:, 512:1024], lg[:, 512:1024], 2.0, rec[:, 512:1024],
                    mybir.AluOpType.subtract, MUL,
                )
                nc.vector.tensor_mul(cj[0:64, :], pv2[0:64, 0:512], rec[:, 0:512])
                nc.vector.scalar_tensor_tensor(
                    cj[64:128, :], pv2[0:64, 512:1024], -1.0, lg[:, 512:1024],
                    MUL, MUL,
                )
            else:
                nc.scalar.activation(lg[:], pv2[64:128, :], LOG)
                nc.scalar.activation(rec[:], lg[:], EXP, scale=-1.0)
                nc.vector.tensor_mul(cj[0:64, :], pv2[0:64, 0:512], rec[:, 0:512])
                nc.vector.tensor_mul(cj[64:128, :], pv2[0:64, 512:1024], rec[:, 512:1024])

        def make_flush(i, pg, qlo, st_, sp_):
            def fl():
                va = vhat[:, 512 * i + 128 * ha : 512 * i + 128 * (ha + 1)]
                vb = vhat[:, 512 * i + 128 * hb : 512 * i + 128 * (hb + 1)]
                nc.tensor.matmul(
                    pv2[:, qlo:512], va, pg[:, qlo:512], start=st_, stop=sp_,
                )
                nc.tensor.matmul(
                    pv2[:, 512 + qlo : 1024], vb, pg[:, 512 + qlo : 1024],
                    start=st_, stop=sp_,
                )
                if sp_:
                    emit_recip()
            return fl

        # tile 0 first (full-width, unmasked: the PSUM-start flush never
        # waits on the gpsimd mask chain), then diagonal (masked) tiles so
        # their exp->mask chain gets CARRY_LAG tiles of slack, then the
        # remaining off-diagonal tiles so the block-end carry drain flushes
        # tiles whose pg needs no gpsimd hop.
        if J == 0:
            order = [0, 1, 2, 3]
        else:
            order = [0] + list(range(4 * J, nk)) + list(range(1, 4 * J))
        for idx, i in enumerate(order):
            dq = i - 4 * J
            qlo = 128 * dq if dq > 0 else 0
            # per-head score tiles: halves the exp latency in the stp ring
            # (DVE exps head a, ACT exps head b, concurrently on every tile)
            sta = stp.tile([128, 512], F32, tag="st", name=f"sta{J}_{hp}_{i}{tag}")
            stb = stp.tile([128, 512], F32, tag="st", name=f"stb{J}_{hp}_{i}{tag}")
            pg = ppool.tile([128, 1024], BF16, tag="pg", name=f"pg{J}_{hp}_{i}{tag}")
            if ST_FP8:
                st_mm(ha, sta[:, qlo:512], i, qlo)
                st_mm(hb, stb[:, qlo:512], i, qlo)
            else:
                nc.tensor.matmul(
                    sta[:, qlo:512],
                    qk[0:64, S + 128 * i : S + 128 * (i + 1)],
                    qk[0:64, 512 * J + qlo : 512 * (J + 1)],
                    start=True, stop=True,
                )
                nc.tensor.matmul(
                    stb[:, qlo:512],
                    qk[64:128, S + 128 * i : S + 128 * (i + 1)],
                    qk[64:128, 512 * J + qlo : 512 * (J + 1)],
                    start=True, stop=True,
                )
            nc.vector.tensor_scalar(
                pg[:, qlo:512].bitcast(I16), sta[:, qlo:512],
                SCH_A / 65536.0, SCH_B / 65536.0, MUL, ADD,
            )
            if idx <= 1:
                # the previous block's reciprocal (~1.8us) heads the ACT
                # queue at block start; the first two tiles' head-b exps go
                # to DVE so the block's first PV flushes never wait on that
                # backlog
                nc.vector.tensor_scalar(
                    pg[:, 512 + qlo : 1024].bitcast(I16), stb[:, qlo:512],
                    SCH_A / 65536.0, SCH_B / 65536.0, MUL, ADD,
                )
            else:
                nc.scalar.activation(
                    pg[:, 512 + qlo : 1024], stb[:, qlo:512], EXP, scale=0.125
                )
            if dq >= 0:
                nc.gpsimd.tensor_mul(pg[:, qlo : qlo + 128], pg[:, qlo : qlo + 128], tri_t[:])
                nc.gpsimd.tensor_mul(
                    pg[:, 512 + qlo : 512 + qlo + 128],
                    pg[:, 512 + qlo : 512 + qlo + 128], tri_t[:],
                )
            if len(carry) >= CARRY_LAG:
                step_carry()
            carry.append(make_flush(i, pg, qlo, idx == 0, idx == nk - 1))
            # weave in pending proj/out work between attention tiles; short
            # blocks pop faster so fewer fillers drain in a burst at block
            # end (right where the next block's exps queue)
            if idx >= 1:
                for _ in range(2 if nk > 8 else 3):
                    if fill:
                        fill.pop(0)()
        drain_carry()
        # anything not absorbed must still be emitted before the next block
        # (the dependency tracker keys on emission order)
        while fill:
            fill.pop(0)()

    def po_mm(po, J, mp):
        for mh in range(2):
            m = 2 * mp + mh
            for t2 in range(2):
                nc.tensor.matmul(
                    po[:, 512 * mh : 512 * (mh + 1)],
                    wo_a[:, 512 * t2 + 128 * m : 512 * t2 + 128 * (m + 1)],
                    ctx_j[J][:, 512 * t2 : 512 * (t2 + 1)],
                    start=(t2 == 0), stop=(t2 == 1),
                )

    def out_fillers(J):
        box = {}
        out3 = outT[512 * J : 512 * (J + 1), :].rearrange("(m p) w -> p m w", m=4)

        def mm(mp):
            def f():
                if "ob" not in box:
                    box["ob"] = opool.tile([128, 2048], BF16, tag="ob", name=f"ob{J}{tag}")
                box[mp] = projp.tile([128, 1024], F32, tag="pj", name=f"po{J}_{mp}{tag}")
                po_mm(box[mp], J, mp)
            return f

        def cp(mp):
            def f():
                ob = box["ob"]
                if mp == 0:
                    nc.scalar.copy(ob[:, 0:1024], box[0][:])
                else:
                    nc.vector.tensor_copy(ob[:, 1024:2048], box[1][:])
            return f

        def dma(mp):
            def f():
                (nc.scalar if mp == 0 else nc.sync).dma_start(
                    out3[:, 2 * mp : 2 * mp + 2, :],
                    box["ob"][:, 1024 * mp : 1024 * (mp + 1)].rearrange("p (m w) -> p m w", m=2),
                )
            return f

        return [mm(0), cp(0), dma(0), mm(1), cp(1), dma(1)]

    def emit_out_last(J):
        # tail is latency-critical: copy halves on both engines in
        # parallel and fire each half's DMA as soon as it lands
        ob = opool.tile([128, 2048], BF16, tag="ob", name=f"ob{J}{tag}")
        out3 = outT[512 * J : 512 * (J + 1), :].rearrange("(m p) w -> p m w", m=4)
        for mp in range(2):
            # mp=1 reuses pv2's pool: its slot frees exactly when the final
            # recip (the tail dependency anyway) finishes reading pv2
            pool = projp if mp == 0 else pvp
            po = pool.tile([128, 1024], F32, tag="pj" if mp == 0 else "pv", name=f"po{J}_{mp}{tag}")
            po_mm(po, J, mp)
            for mh in range(2):
                m = 2 * mp + mh
                cp_ = nc.scalar.copy if mh == 0 else nc.vector.tensor_copy
                cp_(
                    ob[:, 1024 * mp + 512 * mh : 1024 * mp + 512 * (mh + 1)],
                    po[:, 512 * mh : 512 * (mh + 1)],
                )
                (nc.scalar if mh else nc.sync).dma_start(
                    out3[:, m : m + 1, :],
                    ob[:, 1024 * mp + 512 * mh : 1024 * mp + 512 * (mh + 1)].rearrange(
                        "p (m w) -> p m w", m=1
                    ),
                )

    # ---- schedule: J=0 projections run up front (DMA-bound phase); all
    # later proj/V/out work is woven between attention tiles as fillers.
    for f in qk_fillers(0, 0) + qk_fillers(1, 0) + v_fillers(0):
        f()
    fill = []
    for J in range(NJ):
        if J > 0:
            fill += out_fillers(J - 1)
        emit_att(J, 0, fill)
        if J + 1 < NJ:
            fill += v_fillers(J + 1)
            fill += qk_fillers(0, J + 1)
            fill += qk_fillers(1, J + 1)
        emit_att(J, 1, fill)
    # dummy matmuls bridge the final-reciprocal PE gap so the HAM clock
    # gate stays at full rate for the last out-projection matmuls
    wfin = stp.tile([128, 512], F32, tag="st", name=f"wfin{tag}")
    for _ in range(12):
        nc.tensor.matmul(wfin[:], wup[:, 0:128], wup[:], start=True, stop=True)
    emit_out_last(NJ - 1)


def build(repeat=1, dup=frozenset()):
    nc = bass.Bass("TRN2", target_bir_lowering=False, debug=False, num_devices=N_CORES)
    xq = nc.dram_tensor("xq", [512, 2048], BF16, kind="ExternalInput").ap()
    xk = nc.dram_tensor("xk", [512, 2048], BF16, kind="ExternalInput").ap()
    xv = nc.dram_tensor("xv", [512, 2048], BF16, kind="ExternalInput").ap()
    wq = nc.dram_tensor("wq", [128, 1024], BF16, kind="ExternalInput").ap()
    wk = nc.dram_tensor("wk", [128, 1024], BF16, kind="ExternalInput").ap()
    wv = nc.dram_tensor("wv", [128, 1024], BF16, kind="ExternalInput").ap()
    wo = nc.dram_tensor("wo", [128, 1024], BF16, kind="ExternalInput").ap()
    tri = nc.dram_tensor("tri", [128, 128], BF16, kind="ExternalInput").ap()
    outT = nc.dram_tensor("outT", [D * 4, 512], BF16, kind="ExternalOutput").ap()
    aps = (xq, xk, xv, wq, wk, wv, wo, tri, outT)
    with _TC(nc) as tc:
        with ExitStack() as ctx:
            pools = _make_pools(ctx, tc)
            for r in range(repeat):
                _emit_body(nc, tc, aps, pools, tag=(f"_r{r}" if r else ""))
    _fix_sync_waits(nc)
    return nc


def make_in_maps(input_Q, input_K, input_V, W_Q, W_K, W_V, W_O):
    bf = ml_dtypes.bfloat16

    def chunked(xTb):
        # [D, S] -> [4c*128p, (f w)]: chunk c at rows 128c, each partition
        # row holds its full 4KB of (f, w) data contiguously
        a = xTb.reshape(4, 128, 4, 512).transpose(2, 1, 0, 3)
        return np.ascontiguousarray(a.reshape(512, 2048))

    def wfold(wT, f):
        # [f*128, w] -> [128, f*w]: per-partition f-major contiguous
        a = wT.reshape(f, 128, -1).transpose(1, 0, 2)
        return np.ascontiguousarray(a.reshape(128, -1))

    xT = {}
    for b in range(B):
        xT[("q", b)] = chunked(input_Q[b].T.astype(bf))
        xT[("k", b)] = chunked(input_K[b].T.astype(bf))
        xT[("v", b)] = chunked(input_V[b].T.astype(bf))
    wslices = {}
    for g in range(HG):
        rows = slice(256 * g, 256 * (g + 1))
        wslices[("wq", g)] = wfold(np.ascontiguousarray(W_Q[rows, :].T).astype(bf), 4)
        wslices[("wk", g)] = wfold(np.ascontiguousarray(W_K[rows, :].T).astype(bf), 4)
        wslices[("wv", g)] = wfold(np.ascontiguousarray(W_V[rows, :].T).astype(bf), 4)
        wslices[("wo", g)] = wfold(np.ascontiguousarray(W_O[:, rows].T).astype(bf), 2)
    kk, qq = np.meshgrid(np.arange(128), np.arange(128), indexing="ij")
    tri_np = (qq >= kk).astype(bf)
    in_maps = []
    for c in range(N_CORES):
        b, g = c // HG, c % HG
        in_maps.append(
            {
                "xq": xT[("q", b)],
                "xk": xT[("k", b)],
                "xv": xT[("v", b)],
                "wq": wslices[("wq", g)],
                "wk": wslices[("wk", g)],
                "wv": wslices[("wv", g)],
                "wo": wslices[("wo", g)],
                "tri": tri_np,
            }
        )
    return in_maps


_cache = {}


def kernel(**inputs):
    input_Q = np.asarray(inputs["input_Q"], np.float32)
    input_K = np.asarray(inputs["input_K"], np.float32)
    input_V = np.asarray(inputs["input_V"], np.float32)
    W_Q = np.asarray(inputs["W_Q"], np.float32)
    W_K = np.asarray(inputs["W_K"], np.float32)
    W_V = np.asarray(inputs["W_V"], np.float32)
    W_O = np.asarray(inputs["W_O"], np.float32)
    if "nc" not in _cache:
        _cache["nc"] = build()
    nc = _cache["nc"]
    in_maps = make_in_maps(input_Q, input_K, input_V, W_Q, W_K, W_V, W_O)
    res = run_bass_kernel_spmd(nc, in_maps, list(range(N_CORES))).results

    def unblock(a):
        # [D*4, 512] blocks (J, m) -> [D, S]
        return a.reshape(4, 4, 128, 512).transpose(1, 2, 0, 3).reshape(D, S)

    out = np.empty((B, S, D), np.float32)
    for b in range(B):
        out[b] = unblock(res[2 * b]["outT"]).T.astype(np.float32) + unblock(
            res[2 * b + 1]["outT"]
        ).T.astype(np.float32)
    return out

